# revision 6
# baseline (speedup 1.0000x reference)
"""Trainium2 Bass kernel for nn_ClassifierB (embed+GRU+heads), 8-core data-parallel.

Strategy (per core, batch-sharded B=32 -> 4 per core):
  - All matmuls run with the contraction dim on SBUF partitions ("transposed"
    layouts); weights are host-pre-transposed and cast to bf16. Tokens are
    host-reordered t-major: tok = (t, b, p).
  - embed:  xT[h, tok]  = tanh(W_embed . featT + b_embed)   (PE, bf16, fp32 psum)
  - gx:     gxT[3h,tok] = W_ih . xT + (b_ih + b_hh[rz])     (PE) -> DRAM, step-major
  - GRU over T=40 steps, state hT [1024, 48] kept transposed in bf16:
      six 4-m-tile matmul waves per step, one PSUM bank per m-tile
      (start=True clears a whole bank); gx is injected into the rz psum via
      identity-weight matmuls so the sigmoid reads psum directly; n-gate path
      on DVE with per-partition b_hh_n via scalar_tensor_tensor.
  - heads:  action = W_act . hid + b_act; activity = W_activ . maxpool_p(hid)
  - Host reassembles/transposes per-core outputs to the reference layout.
"""

import sys

sys.path.insert(0, "/opt/trn_rl_repo")

import numpy as np
import ml_dtypes

import concourse.bass as bass
import concourse.mybir as mybir
import concourse.tile as tile
from concourse import bacc
from concourse.bass_utils import run_bass_kernel_spmd
from concourse.masks import make_identity

BF16 = mybir.dt.bfloat16
F32 = mybir.dt.float32
AOP = mybir.AluOpType
AFT = mybir.ActivationFunctionType

B, T, P, E, H = 32, 40, 12, 2048, 1024
A_DIM, ACT_DIM = 9, 8
NCORES = 8
BC = B // NCORES          # 4 batch elems per core
S = BC * P                # 48 sequences per core
KE = E // 128             # 16 contraction tiles for embed
KH = H // 128             # 8 contraction tiles for H
M3 = 3 * H // 128         # 24 output tiles of 3H
MH = H // 128             # 8 output tiles of H
NB = np.dtype(ml_dtypes.bfloat16)


def _fap(ap_obj, offset, dims):
    """AP on the same tensor: keep partition dim, replace free dims."""
    return bass.AP(
        tensor=ap_obj.tensor,
        offset=ap_obj.offset + offset,
        ap=[ap_obj.ap[0]] + [list(d) for d in dims],
    )


def _build(t_steps=T):
    NTOK = BC * t_steps * P
    TCH = min(10, t_steps)            # timesteps per token-chunk
    CHUNK = TCH * S                   # tokens per chunk (contig, t-major)
    n_chunks = t_steps // TCH
    nc = bacc.Bacc("TRN2", target_bir_lowering=False, debug=False,
                   num_devices=NCORES)

    # ---- DRAM parameters (host-prepped layouts) ----
    featT_d = nc.declare_dram_parameter("featT", [KE, 128, NTOK], BF16, isOutput=False)
    weT_d = nc.declare_dram_parameter("weT", [KE, 128, H], BF16, isOutput=False)
    bemb_d = nc.declare_dram_parameter("bemb", [128, MH], F32, isOutput=False)
    wihT_d = nc.declare_dram_parameter("wihT", [KH, 128, 3 * H], BF16, isOutput=False)
    biasg_d = nc.declare_dram_parameter("biasg", [128, M3], F32, isOutput=False)
    whhT_d = nc.declare_dram_parameter("whhT", [KH, 128, 3 * H], BF16, isOutput=False)
    bhhn_d = nc.declare_dram_parameter("bhhn", [128, MH], F32, isOutput=False)
    wactT_d = nc.declare_dram_parameter("wactT", [KH, 128, A_DIM], BF16, isOutput=False)
    bact_d = nc.declare_dram_parameter("bact", [A_DIM, 1], F32, isOutput=False)
    wactivT_d = nc.declare_dram_parameter("wactivT", [KH, 128, ACT_DIM], BF16, isOutput=False)
    bactiv_d = nc.declare_dram_parameter("bactiv", [ACT_DIM, 1], F32, isOutput=False)

    out_act_d = nc.declare_dram_parameter(
        "out_action", [A_DIM, NTOK], F32, isOutput=True)
    out_activ_d = nc.declare_dram_parameter(
        "out_activity", [ACT_DIM, t_steps * BC], F32, isOutput=True)

    # internal DRAM bounce for gx, step-major: [t, part, m*s]
    gxs_d = nc.dram_tensor("gxs", [t_steps, 128, M3 * S], BF16)

    with tile.TileContext(nc) as tc:
        pc = tc.alloc_tile_pool(name="consts", bufs=1)
        px = tc.alloc_tile_pool(name="xT", bufs=1)
        pwih = tc.alloc_tile_pool(name="wih", bufs=1)
        pfeat = tc.alloc_tile_pool(name="feat", bufs=1)

        # ---- constants ----
        ident = pc.tile([128, 128], BF16)
        make_identity(nc, ident[:])
        bemb_s = pc.tile([128, MH], F32)
        nc.sync.dma_start(out=bemb_s[:], in_=bemb_d[:])
        biasg_s = pc.tile([128, M3], F32)
        nc.sync.dma_start(out=biasg_s[:], in_=biasg_d[:])
        bhhn_s = pc.tile([128, MH], F32)
        nc.sync.dma_start(out=bhhn_s[:], in_=bhhn_d[:])
        wactT_s = pc.tile([128, KH, A_DIM], BF16)
        nc.sync.dma_start(out=wactT_s[:], in_=wactT_d[:].rearrange("k p a -> p k a"))
        bact_s = pc.tile([A_DIM, 1], F32)
        nc.sync.dma_start(out=bact_s[:], in_=bact_d[:])
        wactivT_s = pc.tile([128, KH, ACT_DIM], BF16)
        nc.sync.dma_start(out=wactivT_s[:], in_=wactivT_d[:].rearrange("k p a -> p k a"))
        bactiv_s = pc.tile([ACT_DIM, 1], F32)
        nc.sync.dma_start(out=bactiv_s[:], in_=bactiv_d[:])

        # ---- phase 0: big weight loads ----
        weT_s = pfeat.tile([128, KE, H], BF16)
        nc.sync.dma_start(out=weT_s[:], in_=weT_d[:].rearrange("k p h -> p k h"))
        wihT_s = pwih.tile([128, KH, 3 * H], BF16)
        nc.sync.dma_start(out=wihT_s[:], in_=wihT_d[:].rearrange("k p g -> p k g"))

        featT_s = pfeat.tile([128, KE, NTOK], BF16)
        for c in range(n_chunks):
            c0 = c * CHUNK
            nc.sync.dma_start(
                out=featT_s[:, :, c0:c0 + CHUNK],
                in_=featT_d[:, :, c0:c0 + CHUNK].rearrange("k p t -> p k t"))

        xT_s = px.tile([128, MH, NTOK], BF16)

        # ---- phase 1: embed ----
        with tc.tile_pool(name="psum_e", bufs=4, space="PSUM") as ppe:
            for c in range(n_chunks):
                c0 = c * CHUNK
                for m in range(MH):
                    ps = ppe.tile([128, CHUNK], F32, tag="pse")
                    for k in range(KE):
                        nc.tensor.matmul(
                            ps[:], weT_s[:, k, m * 128:(m + 1) * 128],
                            featT_s[:, k, c0:c0 + CHUNK],
                            start=(k == 0), stop=(k == KE - 1))
                    nc.scalar.activation(
                        xT_s[:, m, c0:c0 + CHUNK], ps[:], AFT.Tanh,
                        bias=bemb_s[:, m:m + 1])
        pfeat.release()

        # ---- phase 2: gx -> DRAM (step-major) ----
        with tc.tile_pool(name="psum_g", bufs=4, space="PSUM") as ppg, \
             tc.tile_pool(name="stag", bufs=2) as pstag:
            for c in range(n_chunks):
                c0 = c * CHUNK
                stag = pstag.tile([128, TCH, M3 * S], BF16, tag="stag")
                for m in range(M3):
                    ps = ppg.tile([128, CHUNK], F32, tag="psg")
                    for k in range(KH):
                        nc.tensor.matmul(
                            ps[:], wihT_s[:, k, m * 128:(m + 1) * 128],
                            xT_s[:, k, c0:c0 + CHUNK],
                            start=(k == 0), stop=(k == KH - 1))
                    nc.scalar.activation(
                        _fap(stag[:], m * S, [[M3 * S, TCH], [1, S]]),
                        ps[:], AFT.Identity, bias=biasg_s[:, m:m + 1])
                nc.sync.dma_start(
                    out=gxs_d[c * TCH:(c + 1) * TCH, :, :].rearrange(
                        "t p m -> p t m"),
                    in_=stag[:])
        pwih.release()
        px.release()

        # ---- phase 3: GRU scan ----
        phid = tc.alloc_tile_pool(name="hid", bufs=1)
        hidT_s = phid.tile([128, KH, t_steps, S], BF16)
        poolT_s = phid.tile([128, KH, t_steps, BC], BF16)
        pwhh = tc.alloc_tile_pool(name="whh", bufs=1)
        whhT_s = pwhh.tile([128, KH, 3 * H], BF16)
        nc.sync.dma_start(out=whhT_s[:], in_=whhT_d[:].rearrange("k p g -> p k g"))
        pstep = tc.alloc_tile_pool(name="step", bufs=2)

        with tc.tile_pool(name="psum_r", bufs=2, space="PSUM") as ppr:
            for t in range(t_steps):
                gxt = pstep.tile([128, M3, S], BF16, tag="gxt", bufs=6)
                nc.sync.dma_start(out=gxt[:], in_=gxs_d[t, :, :])
                rz_t = pstep.tile([128, 16, S], BF16, tag="rz")
                B_t = pstep.tile([128, MH, S], BF16, tag="B")
                np_t = pstep.tile([128, MH, S], F32, tag="np")
                n_t = pstep.tile([128, MH, S], BF16, tag="n")
                if t > 0:
                    A_t = pstep.tile([128, MH, S], F32, tag="A")

                if t == 0:
                    # h0 = 0: no matmuls. rz = sigmoid(gx_rz)
                    nc.scalar.activation(rz_t[:], gxt[:, 0:16, :], AFT.Sigmoid)
                    for j in range(MH):
                        # n'' = (r_j * bhh_n_j) + gx_n_j
                        nc.vector.scalar_tensor_tensor(
                            np_t[:, j, :], rz_t[:, j, :], bhhn_s[:, j:j + 1],
                            gxt[:, 16 + j, :], AOP.mult, AOP.add)
                    nc.scalar.activation(n_t[:], np_t[:], AFT.Tanh)
                    nc.vector.tensor_scalar(B_t[:], rz_t[:, 8:16, :], -1.0, 1.0,
                                            AOP.mult, AOP.add)
                    nc.vector.tensor_mul(hidT_s[:, :, 0, :], B_t[:], n_t[:])
                else:
                    # 6 matmul waves, each 4 m-tiles, one PSUM bank per m-tile
                    ps_w = {}
                    for kind, hf in (("r", 0), ("z", 0), ("n", 0),
                                     ("r", 1), ("z", 1), ("n", 1)):
                        ps = ppr.tile([128, 4, 512], F32, tag="psr")
                        m0 = {"r": 0, "z": 8, "n": 16}[kind] + 4 * hf
                        if kind != "n":
                            for ml in range(4):
                                nc.tensor.matmul(
                                    ps[:, ml, :S], ident[:],
                                    gxt[:, m0 + ml, :],
                                    start=True, stop=False)
                        for k in range(KH):
                            for ml in range(4):
                                nc.tensor.matmul(
                                    ps[:, ml, :S],
                                    whhT_s[:, k, (m0 + ml) * 128:(m0 + ml + 1) * 128],
                                    hidT_s[:, k, t - 1, :],
                                    start=(kind == "n" and k == 0),
                                    stop=(k == KH - 1))
                        ps_w[(kind, hf)] = ps
                    for hf in range(2):
                        j0 = 4 * hf
                        nc.scalar.activation(
                            rz_t[:, j0:j0 + 4, :],
                            _fap(ps_w[("r", hf)][:], 0, [[512, 4], [1, S]]),
                            AFT.Sigmoid)
                        nc.scalar.activation(
                            rz_t[:, 8 + j0:12 + j0, :],
                            _fap(ps_w[("z", hf)][:], 0, [[512, 4], [1, S]]),
                            AFT.Sigmoid)
                        # A = z * h_prev ; B = 1 - z
                        nc.vector.tensor_mul(
                            A_t[:, j0:j0 + 4, :], rz_t[:, 8 + j0:12 + j0, :],
                            hidT_s[:, j0:j0 + 4, t - 1, :])
                        nc.vector.tensor_scalar(
                            B_t[:, j0:j0 + 4, :], rz_t[:, 8 + j0:12 + j0, :],
                            -1.0, 1.0, AOP.mult, AOP.add)
                        # n' = (hn + bhh_n) * r   (per m-tile, psum src)
                        for j in range(j0, j0 + 4):
                            nc.vector.scalar_tensor_tensor(
                                np_t[:, j, :], ps_w[("n", hf)][:, j - j0, :S],
                                bhhn_s[:, j:j + 1], rz_t[:, j, :],
                                AOP.add, AOP.mult)
                        # n'' = n' + gx_n
                        nc.vector.tensor_add(
                            np_t[:, j0:j0 + 4, :], np_t[:, j0:j0 + 4, :],
                            gxt[:, 16 + j0:20 + j0, :])
                        nc.scalar.activation(
                            n_t[:, j0:j0 + 4, :], np_t[:, j0:j0 + 4, :],
                            AFT.Tanh)
                        # h_new = B*n + A  -> hidT (bf16)
                        nc.vector.tensor_mul(
                            np_t[:, j0:j0 + 4, :], B_t[:, j0:j0 + 4, :],
                            n_t[:, j0:j0 + 4, :])
                        nc.vector.tensor_add(
                            hidT_s[:, j0:j0 + 4, t, :], np_t[:, j0:j0 + 4, :],
                            A_t[:, j0:j0 + 4, :])
                # max-pool over persons (innermost 12)
                hview = _fap(hidT_s[:], t * S,
                             [[t_steps * S, KH], [P, BC], [1, P]])
                nc.vector.tensor_reduce(
                    poolT_s[:, :, t, :], hview, op=AOP.max,
                    axis=mybir.AxisListType.X)

        pstep.release()
        pwhh.release()

        # ---- phase 4: heads ----
        pout = tc.alloc_tile_pool(name="outs", bufs=1)
        act_s = pout.tile([A_DIM, NTOK], F32)
        activ_s = pout.tile([ACT_DIM, t_steps * BC], F32)
        with tc.tile_pool(name="psum_o", bufs=2, space="PSUM") as ppo:
            ACH = min(512 // S * S, NTOK)
            for c0 in range(0, NTOK, ACH):
                cw = min(ACH, NTOK - c0)
                ps = ppo.tile([A_DIM, ACH], F32, tag="psa")
                for k in range(KH):
                    rhs = _fap(hidT_s[:, k, 0, :], c0, [[1, cw]])
                    nc.tensor.matmul(ps[:, :cw], wactT_s[:, k, :], rhs,
                                     start=(k == 0), stop=(k == KH - 1))
                nc.scalar.activation(act_s[:, c0:c0 + cw], ps[:, :cw],
                                     AFT.Identity, bias=bact_s[:])
            ps2 = ppo.tile([ACT_DIM, t_steps * BC], F32, tag="psv")
            for k in range(KH):
                rhs = _fap(poolT_s[:, k, 0, :], 0, [[1, t_steps * BC]])
                nc.tensor.matmul(ps2[:], wactivT_s[:, k, :], rhs,
                                 start=(k == 0), stop=(k == KH - 1))
            nc.scalar.activation(activ_s[:], ps2[:], AFT.Identity,
                                 bias=bactiv_s[:])
        nc.sync.dma_start(out=out_act_d[:], in_=act_s[:])
        nc.sync.dma_start(out=out_activ_d[:], in_=activ_s[:])
        pout.release()
        phid.release()
        pc.release()

    nc.compile()
    return nc


_CACHE = {}


def _get_nc(t_steps=T):
    if t_steps not in _CACHE:
        _CACHE[t_steps] = _build(t_steps)
    return _CACHE[t_steps]


def _prep_shared(W_embed, b_embed, W_ih, W_hh, b_ih, b_hh,
                 W_act, b_act, W_activ, b_activ):
    d = {}
    d["weT"] = np.ascontiguousarray(
        W_embed.T.reshape(KE, 128, H)).astype(NB)
    d["bemb"] = np.ascontiguousarray(
        b_embed.reshape(MH, 128).T).astype(np.float32)
    d["wihT"] = np.ascontiguousarray(
        W_ih.T.reshape(KH, 128, 3 * H)).astype(NB)
    bg = b_ih.astype(np.float64).copy()
    bg[:2 * H] += b_hh[:2 * H]
    d["biasg"] = np.ascontiguousarray(
        bg.reshape(M3, 128).T).astype(np.float32)
    d["whhT"] = np.ascontiguousarray(
        W_hh.T.reshape(KH, 128, 3 * H)).astype(NB)
    d["bhhn"] = np.ascontiguousarray(
        b_hh[2 * H:].reshape(MH, 128).T).astype(np.float32)
    d["wactT"] = np.ascontiguousarray(
        W_act.T.reshape(KH, 128, A_DIM)).astype(NB)
    d["bact"] = b_act.reshape(A_DIM, 1).astype(np.float32)
    d["wactivT"] = np.ascontiguousarray(
        W_activ.T.reshape(KH, 128, ACT_DIM)).astype(NB)
    d["bactiv"] = b_activ.reshape(ACT_DIM, 1).astype(np.float32)
    return d


def kernel(feature, W_embed, b_embed, W_ih, W_hh, b_ih, b_hh,
           W_act, b_act, W_activ, b_activ, _t_steps=None):
    feature = np.asarray(feature, dtype=np.float32)
    t_steps = feature.shape[1] if _t_steps is None else _t_steps
    ntok = BC * t_steps * P
    nc = _get_nc(t_steps)
    shared = _prep_shared(
        np.asarray(W_embed, np.float32), np.asarray(b_embed, np.float32),
        np.asarray(W_ih, np.float32), np.asarray(W_hh, np.float32),
        np.asarray(b_ih, np.float32), np.asarray(b_hh, np.float32),
        np.asarray(W_act, np.float32), np.asarray(b_act, np.float32),
        np.asarray(W_activ, np.float32), np.asarray(b_activ, np.float32))

    in_maps = []
    for c in range(NCORES):
        # t-major token order: tok = (t, b, p)
        fc = feature[c * BC:(c + 1) * BC].transpose(1, 0, 2, 3).reshape(ntok, E)
        featT = np.ascontiguousarray(fc.T.reshape(KE, 128, ntok)).astype(NB)
        m = dict(shared)
        m["featT"] = featT
        in_maps.append(m)

    res = run_bass_kernel_spmd(nc, in_maps, list(range(NCORES)))

    act_parts, activ_parts = [], []
    for c in range(NCORES):
        a = res.results[c]["out_action"]          # [9, ntok] tok=(t,b,p)
        act_parts.append(
            a.reshape(A_DIM, t_steps, BC, P).transpose(2, 1, 3, 0))
        v = res.results[c]["out_activity"]        # [8, t*bc]
        activ_parts.append(
            v.reshape(ACT_DIM, t_steps, BC).transpose(2, 1, 0))
    action_logits = np.concatenate(act_parts, 0).reshape(-1, A_DIM)
    activity_logits = np.concatenate(activ_parts, 0).reshape(-1, ACT_DIM)
    return action_logits, activity_logits
